# revision 5
# baseline (speedup 1.0000x reference)
"""Trainium2 Bass kernel for nn_Decoder_17076789969159 (gnn_message_passing).

Sharding: data-parallel over batch (2 groups of 4 cores); within a group the
permuted point axis of each space-filling-curve order is split in 4 contiguous
chunks. Per-order conv outputs are AllGather'd inside the group; the next
layer gathers its inputs with host-composed inverse-permutation indices, so
no scatter is ever needed on device.

Row gathers use the SWDGE dma_gather extended instruction: one call per
(tile, slab) with 512-640 int16 indices. transpose=True emits the gathered
rows channel-major directly, eliminating the PE-transpose + copy chains.

Self-contained: hardcodes all shapes from the problem spec.
"""

import os
import numpy as np
import ml_dtypes

BF16 = ml_dtypes.bfloat16

# Problem shapes (hardcoded per contract)
B, N, LL, O, KK, PAD = 2, 32768, 8192, 3, 9, 4
C = 256          # conv output channels
C1R = 304        # conv1 input channels (256 xi + 48 low)
CH1 = 384        # padded h row (3 * 128)
CLS = 13
NCORES, GRP = 8, 4
QN = N // GRP            # 8192 permuted positions per core per order
NPT = 512                # points per conv tile
NT = QN // NPT           # 16 conv tiles per (order) per core
NTS0 = N // NPT          # 64 stage0 tiles (full batch, replicated in group)
NTF = QN // NPT          # 16 final tiles (core's original-index quarter)
NG = NPT + 2 * PAD       # 520 needed gather cols
KGN = 640                # gathered rows per conv tile (125*128 pad)
KGW = KGN // 16          # idx cols per conv tile (wrapped int16 layout)
FW = NPT // 16           # idx cols per 512-row gather
EPS = 1e-5

_CACHE = {}


# ----------------------------------------------------------------------------
# host-side preparation
# ----------------------------------------------------------------------------

def _wrap16(vals):
    """index vector of length 16*S -> [128, S] int16 wrapped layout
    (index i at partition i%16, col i//16; the 16-row pattern replicated
    across all 8 partition stripes for the 8 Q7 gpsimd cores)."""
    v = np.asarray(vals, np.int16)
    S = len(v) // 16
    out = np.empty((128, S), np.int16)
    out[:16] = v.reshape(S, 16).T
    for k in range(1, 8):
        out[16 * k:16 * (k + 1)] = out[:16]
    return out


def _bn_affine(g, b, m, v):
    s = g / np.sqrt(v + EPS)
    return s.astype(np.float32), (b - m * s).astype(np.float32)


def _prep_shared(inp):
    sh = {}
    w1 = np.asarray(inp["w1_w"], np.float32)   # [256, 304, 9]
    w1p = np.zeros((128, KK * 3 * C), np.float32)
    for j in range(KK):
        for kc in range(3):
            ci0 = kc * 128
            ncid = min(128, C1R - ci0)
            if ncid > 0:
                blk = w1[:, ci0:ci0 + ncid, j].T  # [ncid, 256]
                w1p[:ncid, (j * 3 + kc) * C:(j * 3 + kc) * C + C] = blk
    sh["w1p"] = w1p.astype(BF16)

    w2 = np.asarray(inp["w2_w"], np.float32)   # [256, 256, 9]
    w2p = np.zeros((128, KK * 2 * C), np.float32)
    for j in range(KK):
        for kc in range(2):
            blk = w2[:, kc * 128:(kc + 1) * 128, j].T
            w2p[:, (j * 2 + kc) * C:(j * 2 + kc) * C + C] = blk
    sh["w2p"] = w2p.astype(BF16)

    sh["c1"] = np.asarray(inp["conv1_w"], np.float32).T.astype(BF16)  # [128,48]

    ow = np.asarray(inp["out_w"], np.float32)  # [13, 256]
    owp = np.zeros((128, 2 * CLS), np.float32)
    for g in range(2):
        owp[:, g * CLS:(g + 1) * CLS] = ow[:, g * 128:(g + 1) * 128].T
    sh["outw"] = owp.astype(BF16)

    sh["eye"] = np.eye(128, dtype=BF16)

    bnv = np.zeros((128, 11), np.float32)
    s1, b1 = _bn_affine(inp["bn1_g"], inp["bn1_b"], inp["bn1_m"], inp["bn1_v"])
    bnv[:48, 0], bnv[:48, 1] = s1, b1
    sc, bc = _bn_affine(inp["bnc1_g"], inp["bnc1_b"], inp["bnc1_m"], inp["bnc1_v"])
    bc = bc + np.asarray(inp["w1_b"], np.float32) * sc
    for g in range(2):
        bnv[:, 2 + g] = sc[g * 128:(g + 1) * 128] / 3.0
        bnv[:, 4 + g] = bc[g * 128:(g + 1) * 128]
    sc2, bc2 = _bn_affine(inp["bnc2_g"], inp["bnc2_b"], inp["bnc2_m"], inp["bnc2_v"])
    bc2 = bc2 + np.asarray(inp["w2_b"], np.float32) * sc2
    for g in range(2):
        bnv[:, 6 + g] = sc2[g * 128:(g + 1) * 128] / 3.0
        bnv[:, 8 + g] = bc2[g * 128:(g + 1) * 128]
    bnv[:CLS, 10] = np.asarray(inp["out_b"], np.float32)
    sh["bnvec"] = bnv

    # interp index tables: per chunk two 512-row gathers (i0 rows, i1 rows)
    pos = np.arange(N, dtype=np.float64) * ((LL - 1) / (N - 1))
    i0 = np.floor(pos).astype(np.int64)
    i1 = np.minimum(i0 + 1, LL - 1)
    t = (pos - i0).astype(np.float32)
    icols = []
    for ch in range(NTS0):
        s = slice(ch * NPT, (ch + 1) * NPT)
        icols.append(_wrap16(i0[s]))
        icols.append(_wrap16(i1[s]))
    sh["iidx"] = np.concatenate(icols, axis=1)           # [128, NTS0*2*FW]
    tt = np.zeros((128, NTS0 * 4), np.float32)
    for ch in range(NTS0):
        for s in range(4):
            tt[:, ch * 4 + s] = t[ch * NPT + s * 128: ch * NPT + (s + 1) * 128]
    sh["tt"] = tt
    return sh


def _prep_core(inp, c):
    b, q = c // GRP, c % GRP
    pc = {}
    x = np.asarray(inp["x"], np.float32)
    pc["xt"] = np.ascontiguousarray(x[b].T).astype(BF16)          # [8192, 256]
    pc["llf"] = np.asarray(inp["low_level_feat"], np.float32)[b].astype(BF16)

    rot = np.asarray(inp["rotations"], np.int64)[:, b, :]          # [O, N]
    inv = np.stack([np.argsort(rot[o], kind="stable") for o in range(O)])

    # geometry weights in permuted space, OOB taps zeroed
    coords = np.asarray(inp["coords"], np.float32)[b]              # [3, N]
    dist = np.asarray(inp["distances"], np.float32)[b]             # [O, N]
    wall = np.zeros((O, KK, N), np.float32)
    ar = np.arange(N)
    for o in range(O):
        co = coords[:, rot[o]]                                     # [3, N]
        d = dist[o]
        dp = np.pad(d, (PAD, PAD))
        cp = np.pad(co, ((0, 0), (PAD, PAD)))
        for j in range(KK):
            dd = (dp[j:j + N] - d) ** 2
            dc = ((cp[:, j:j + N] - co) ** 2).sum(0)
            w = np.exp(-(dd + dc))
            pin = ar + j - PAD
            w[(pin < 0) | (pin >= N)] = 0.0
            wall[o, j] = w
    # per-core chunk-blocked, pre-broadcast to 128 partitions
    wgt = np.zeros((O * NT, KK * NPT), np.float32)
    for o in range(O):
        for tch in range(NT):
            base = q * QN + tch * NPT
            wgt[o * NT + tch] = wall[o, :, base:base + NPT].reshape(KK * NPT)
    pc["wgt"] = np.ascontiguousarray(
        np.broadcast_to(wgt.astype(BF16)[:, None, :], (O * NT, 128, KK * NPT)))

    # L1/L2 gather indices (640-row windows with halo; filler cols -> row 0)
    gcols, ccols = [], []
    mar = np.arange(KGN)
    for o in range(O):
        for tch in range(NT):
            base = q * QN + tch * NPT
            pp = base - PAD + mar
            valid = (pp >= 0) & (pp < N) & (mar < NG)
            ppc = np.clip(pp, 0, N - 1)
            gcols.append(_wrap16(np.where(valid, rot[o][ppc], 0)))
            for os_ in range(O):
                ccols.append(_wrap16(np.where(valid, inv[os_][rot[o][ppc]], 0)))
    pc["gidx"] = np.concatenate(gcols, axis=1)    # [128, O*NT*KGW]
    pc["cidx"] = np.concatenate(ccols, axis=1)    # [128, O*NT*O*KGW]

    fcols = []
    for tch in range(NTF):
        base = q * QN + tch * NPT
        for os_ in range(O):
            fcols.append(_wrap16(inv[os_][base:base + NPT]))
    pc["fidx"] = np.concatenate(fcols, axis=1)    # [128, NTF*O*FW]
    return pc


# ----------------------------------------------------------------------------
# device program
# ----------------------------------------------------------------------------

def _build_nc():
    import concourse.bacc as bacc
    import concourse.bass as bass
    import concourse.tile as tile
    import concourse.mybir as mybir

    dt = mybir.dt
    AF = mybir.ActivationFunctionType
    nocc = os.environ.get("KNOCC", "0") == "1"
    nc = bacc.Bacc("TRN2", target_bir_lowering=False, debug=False,
                   num_devices=1 if nocc else NCORES)

    def EIN(name, shape, dty):
        return nc.dram_tensor(name, list(shape), dty, kind="ExternalInput")

    xt = EIN("xt", [LL, C], dt.bfloat16)
    llf = EIN("llf", [128, N], dt.bfloat16)
    w1p = EIN("w1p", [128, KK * 3 * C], dt.bfloat16)
    w2p = EIN("w2p", [128, KK * 2 * C], dt.bfloat16)
    c1 = EIN("c1", [128, 48], dt.bfloat16)
    outw = EIN("outw", [128, 2 * CLS], dt.bfloat16)
    eye = EIN("eye", [128, 128], dt.bfloat16)
    bnvec = EIN("bnvec", [128, 11], dt.float32)
    wgt = EIN("wgt", [O * NT, 128, KK * NPT], dt.bfloat16)
    gidx = EIN("gidx", [128, O * NT * KGW], dt.int16)
    cidx = EIN("cidx", [128, O * NT * O * KGW], dt.int16)
    fidx = EIN("fidx", [128, NTF * O * FW], dt.int16)
    iidx = EIN("iidx", [128, NTS0 * 2 * FW], dt.int16)
    tt = EIN("tt", [128, NTS0 * 4], dt.float32)

    out = nc.dram_tensor("out", [CLS, QN], dt.float32, kind="ExternalOutput")
    dbg = os.environ.get("KDBG", "0") == "1"
    if dbg:
        dbg_h = nc.dram_tensor("dbg_h", [N, CH1], dt.bfloat16, kind="ExternalOutput")
        dbg_y1in0 = nc.dram_tensor("dbg_y1in0", [QN, C], dt.bfloat16, kind="ExternalOutput")
        dbg_y1all0 = nc.dram_tensor("dbg_y1all0", [N, C], dt.bfloat16, kind="ExternalOutput")
        dbg_y2in0 = nc.dram_tensor("dbg_y2in0", [QN, C], dt.bfloat16, kind="ExternalOutput")

    RG = [[0, 1, 2, 3], [4, 5, 6, 7]]

    with tile.TileContext(nc) as tc:
        with (
            tc.tile_pool(name="dram", bufs=1, space="DRAM") as dpool,
            tc.tile_pool(name="res", bufs=1) as res,
            tc.tile_pool(name="wk", bufs=2) as wk,
            tc.tile_pool(name="wc", bufs=2) as wc,
            tc.tile_pool(name="ps", bufs=4, space="PSUM") as psp,
            tc.tile_pool(name="pt", bufs=4, space="PSUM") as ptp,
        ):
            h_t = dpool.tile([N, CH1], dt.bfloat16, tag="h")
            y1in = [dpool.tile([QN, C], dt.bfloat16, tag=f"y1in{o}",
                               name=f"y1in{o}") for o in range(O)]
            y1all = [dpool.tile([N, C], dt.bfloat16, tag=f"y1all{o}",
                                name=f"y1all{o}") for o in range(O)]
            y2in = [dpool.tile([QN, C], dt.bfloat16, tag=f"y2in{o}",
                               name=f"y2in{o}") for o in range(O)]
            y2all = [dpool.tile([N, C], dt.bfloat16, tag=f"y2all{o}",
                                name=f"y2all{o}") for o in range(O)]

            # resident SBUF constants
            def LOAD(src, shape, dty, tag):
                tl = res.tile(shape, dty, tag=tag, name=tag)
                nc.sync.dma_start(tl[:], src[:])
                return tl
            w1s = LOAD(w1p, [128, KK * 3 * C], dt.bfloat16, "w1s")
            w2s = LOAD(w2p, [128, KK * 2 * C], dt.bfloat16, "w2s")
            c1s = LOAD(c1, [128, 48], dt.bfloat16, "c1s")
            ows = LOAD(outw, [128, 2 * CLS], dt.bfloat16, "ows")
            eys = LOAD(eye, [128, 128], dt.bfloat16, "eys")
            bns = LOAD(bnvec, [128, 11], dt.float32, "bns")
            gis = LOAD(gidx, [128, O * NT * KGW], dt.int16, "gis")
            cis = LOAD(cidx, [128, O * NT * O * KGW], dt.int16, "cis")
            fis = LOAD(fidx, [128, NTF * O * FW], dt.int16, "fis")
            iis = LOAD(iidx, [128, NTS0 * 2 * FW], dt.int16, "iis")
            tts = LOAD(tt, [128, NTS0 * 4], dt.float32, "tts")

            def rows_pm(dram_tile, base, nrows, rowlen):
                """point-major SBUF tile [128, nrows//128, rowlen] <-> dram rows."""
                return bass.AP(dram_tile.tensor, base * rowlen,
                               [[rowlen, 128], [128 * rowlen, nrows // 128],
                                [1, rowlen]])

            # ---------------- stage 0: build h ----------------
            for ch in range(NTS0):
                xg0 = wk.tile([128, 4, C], dt.bfloat16, tag="xg0")
                xg1 = wk.tile([128, 4, C], dt.bfloat16, tag="xg1")
                nc.gpsimd.dma_gather(xg0[:], xt[:, :],
                                     iis[:, ch * 2 * FW:(ch * 2 + 1) * FW],
                                     NPT, NPT, C)
                nc.gpsimd.dma_gather(xg1[:], xt[:, :],
                                     iis[:, (ch * 2 + 1) * FW:(ch * 2 + 2) * FW],
                                     NPT, NPT, C)
                xd = wk.tile([128, 4, C], dt.float32, tag="xd")
                nc.vector.tensor_sub(xd[:], xg1[:], xg0[:])
                xm = wk.tile([128, 4, C], dt.float32, tag="xm")
                for s in range(4):
                    nc.scalar.activation(xm[:, s, :], xd[:, s, :], AF.Copy,
                                         scale=tts[:, ch * 4 + s:ch * 4 + s + 1])
                hrow = wk.tile([128, 4, CH1], dt.bfloat16, tag="hrow")
                nc.vector.tensor_add(hrow[:, :, 0:C], xg0[:], xm[:])

                lsb = wk.tile([128, NPT], dt.bfloat16, tag="lsb")
                nc.sync.dma_start(lsb[:], llf[:, ch * NPT:(ch + 1) * NPT])
                p48 = psp.tile([48, NPT], dt.float32, tag="pc")
                nc.tensor.matmul(p48[:], c1s[:], lsb[:], start=True, stop=True)
                low = wk.tile([48, NPT], dt.bfloat16, tag="low")
                nc.scalar.activation(low[:], p48[:], AF.Relu,
                                     bias=bns[:48, 1:2], scale=bns[:48, 0:1])
                for s in range(4):
                    ptt = ptp.tile([128, 48], dt.bfloat16, tag="pt")
                    nc.tensor.transpose(ptt[:], low[:48, s * 128:(s + 1) * 128],
                                        eys[:48, :48])
                    nc.scalar.activation(hrow[:, s, C:C + 48], ptt[:], AF.Copy)
                nc.vector.memset(hrow[:, :, C + 48:CH1], 0)
                nc.sync.dma_start(rows_pm(h_t, ch * NPT, NPT, CH1), hrow[:])

            # ---------------- conv layer helper ----------------
            def conv_layer(yin, wsb_pack, nkc, gather_one, after_order=None):
                for o in range(O):
                    for tch in range(NT):
                        blk = o * NT + tch
                        hx = gather_one(o, tch)
                        wsb = wc.tile([128, KK * NPT], dt.bfloat16, tag="wsb")
                        nc.sync.dma_start(wsb[:], wgt[blk, :, :])
                        pg = [psp.tile([128, NPT], dt.float32, tag="pc",
                                       name=f"pg{g}") for g in range(2)]
                        for j in range(KK):
                            xw = wk.tile([128, nkc, NPT], dt.bfloat16, tag="xw")
                            for kc in range(nkc):
                                nc.vector.tensor_mul(
                                    xw[:, kc, :], hx[:, kc, j:j + NPT],
                                    wsb[:, j * NPT:(j + 1) * NPT])
                            for g in range(2):
                                for kc in range(nkc):
                                    wsl = wsb_pack[:, ((j * nkc + kc) * C + g * 128):
                                                   ((j * nkc + kc) * C + g * 128 + 128)]
                                    nc.tensor.matmul(
                                        pg[g][:], wsl, xw[:, kc, :],
                                        start=(j == 0 and kc == 0),
                                        stop=(j == KK - 1 and kc == nkc - 1))
                        ysb = wk.tile([128, 2, NPT], dt.bfloat16, tag="ysb")
                        for g in range(2):
                            nc.scalar.activation(ysb[:, g, :], pg[g][:], AF.Copy)
                        yT = wk.tile([128, 4, C], dt.bfloat16, tag="yT")
                        for g in range(2):
                            for s in range(4):
                                ptt = ptp.tile([128, 128], dt.bfloat16, tag="pt")
                                nc.tensor.transpose(
                                    ptt[:], ysb[:, g, s * 128:(s + 1) * 128], eys[:])
                                nc.scalar.activation(
                                    yT[:, s, g * 128:(g + 1) * 128], ptt[:], AF.Copy)
                        nc.sync.dma_start(rows_pm(yin[o], tch * NPT, NPT, C), yT[:])
                    if after_order is not None:
                        after_order(o)

            # L1: one transposing dma_gather per tile -> channel-major [128,3,640]
            def gather_l1(o, tch):
                blk = o * NT + tch
                g1 = wk.tile([128, 3, KGN], dt.bfloat16, tag="g1")
                nc.gpsimd.dma_gather(g1[:], h_t[:, :],
                                     gis[:, blk * KGW:(blk + 1) * KGW],
                                     KGN, KGN, CH1, transpose=True)
                return g1

            def ag1(o):
                if nocc:
                    return
                nc.gpsimd.collective_compute(
                    "AllGather", mybir.AluOpType.bypass, replica_groups=RG,
                    ins=[y1in[o].opt()], outs=[y1all[o].opt()])

            conv_layer(y1in, w1s, 3, gather_l1, after_order=ag1)

            # L2: three transposing dma_gathers (one per order slab), sum, bn+relu
            def gather_l2(o, tch):
                blk = (o * NT + tch) * O
                gs = []
                for os_ in range(O):
                    g2 = wk.tile([128, 2, KGN], dt.bfloat16, tag=f"g2{os_}",
                                 name=f"g2{os_}")
                    nc.gpsimd.dma_gather(g2[:], y1all[os_][:, :],
                                         cis[:, (blk + os_) * KGW:(blk + os_ + 1) * KGW],
                                         KGN, KGN, C, transpose=True)
                    gs.append(g2)
                s12 = wk.tile([128, 2, KGN], dt.float32, tag="s12")
                nc.vector.tensor_add(s12[:], gs[0][:], gs[1][:])
                nc.vector.tensor_add(s12[:], s12[:], gs[2][:])
                hx = wk.tile([128, 2, KGN], dt.bfloat16, tag="hx")
                for g in range(2):
                    nc.scalar.activation(hx[:, g, :], s12[:, g, :], AF.Relu,
                                         bias=bns[:, 4 + g:5 + g],
                                         scale=bns[:, 2 + g:3 + g])
                return hx

            def ag2(o):
                if nocc:
                    return
                nc.gpsimd.collective_compute(
                    "AllGather", mybir.AluOpType.bypass, replica_groups=RG,
                    ins=[y2in[o].opt()], outs=[y2all[o].opt()])

            conv_layer(y2in, w2s, 2, gather_l2, after_order=ag2)

            if dbg:
                nc.sync.dma_start(dbg_h[:, :], h_t[:, :])
                nc.sync.dma_start(dbg_y1in0[:, :], y1in[0][:, :])
                nc.sync.dma_start(dbg_y1all0[:, :], y1all[0][:, :])
                nc.sync.dma_start(dbg_y2in0[:, :], y2in[0][:, :])

            # ---------------- final: bn2+relu+proj ----------------
            for tch in range(NTF):
                gs = []
                for os_ in range(O):
                    blk = tch * O + os_
                    g3 = wk.tile([128, 2, NPT], dt.bfloat16, tag=f"g3{os_}",
                                 name=f"g3{os_}")
                    nc.gpsimd.dma_gather(g3[:], y2all[os_][:, :],
                                         fis[:, blk * FW:(blk + 1) * FW],
                                         NPT, NPT, C, transpose=True)
                    gs.append(g3)
                s3 = wk.tile([128, 2, NPT], dt.float32, tag="s3")
                nc.vector.tensor_add(s3[:], gs[0][:], gs[1][:])
                nc.vector.tensor_add(s3[:], s3[:], gs[2][:])
                h2 = wk.tile([128, 2, NPT], dt.bfloat16, tag="h2")
                for g in range(2):
                    nc.scalar.activation(h2[:, g, :], s3[:, g, :], AF.Relu,
                                         bias=bns[:, 8 + g:9 + g],
                                         scale=bns[:, 6 + g:7 + g])
                pf = psp.tile([CLS, NPT], dt.float32, tag="pc")
                for g in range(2):
                    nc.tensor.matmul(pf[:], ows[:, g * CLS:(g + 1) * CLS],
                                     h2[:, g, :], start=(g == 0), stop=(g == 1))
                osb = wk.tile([CLS, NPT], dt.float32, tag="osb")
                nc.vector.tensor_scalar_add(osb[:], pf[:], bns[:CLS, 10:11])
                nc.sync.dma_start(out[:, tch * NPT:(tch + 1) * NPT], osb[:])

    nc.compile()
    return nc


# ----------------------------------------------------------------------------
# entry point
# ----------------------------------------------------------------------------

def kernel(**inputs):
    from concourse.bass_utils import run_bass_kernel_spmd

    if "nc" not in _CACHE:
        _CACHE["nc"] = _build_nc()
    nc = _CACHE["nc"]

    sh = _prep_shared(inputs)
    in_maps = []
    for c in range(NCORES):
        m = dict(sh)
        m.update(_prep_core(inputs, c))
        in_maps.append(m)

    res = run_bass_kernel_spmd(nc, in_maps, core_ids=list(range(NCORES)))
    outs = res.results
    full = np.zeros((B, CLS, N), np.float32)
    for c in range(NCORES):
        b, q = c // GRP, c % GRP
        full[b, :, q * QN:(q + 1) * QN] = outs[c]["out"]
    return full
